# revision 6
# baseline (speedup 1.0000x reference)
"""Trainium2 Bass kernel for nn_AttentionBlock (GroupNorm + 8-head self-attention + proj + residual).

Full inputs in, full output out. Sharding: 8 cores = 2 batches x 4-way split of
the 4096 query pixels. Each core runs an identical SPMD program on per-core
input data:
  - x for its batch, pixel axis rotated so the core's 1024 query rows sit at
    columns 0..1023 (attention and groupnorm are permutation-invariant over
    keys/pixels, so rotation is exact),
  - shared (host-transposed, bf16) weight matrices.

Device-side math (exact up to fp rounding):
  groupnorm:  h = a (.) x + b   with per-channel a = gn_w * rstd(group),
              b = gn_b - mu(group) * a, computed on-device from bf16 x.
  The affine is folded into the QKV weights on the device:
              q = (Wq diag(a)) x + (Wq b + wq_b),
              k = (Wk diag(a)) x        (k's constant shifts every score row
                                         by a per-query constant -> cancels in
                                         softmax, dropped),
              v = (Wv diag(a)) x        (v's constant vb = Wv b + wv_b rides
                                         through softmax unchanged; it is
                                         applied at the end via
                                         cout = proj_w vb + proj_b).
  attention:  S^T tiles (keys on partitions) = k_tile^T-slices x q; exp via
              ScalarE with the 1/8 scale fused; denominators from a ones-column
              matmul (PV col-packed per head pair); o_norm = oA * recip(oB).
  out:        y^T = proj_w o_norm^T + cout + residual.
"""

import numpy as np
import ml_dtypes
from contextlib import ExitStack

import concourse.bacc as bacc
import concourse.tile as tile
import concourse.mybir as mybir
from concourse.bass_utils import run_bass_kernel_spmd

BF16 = ml_dtypes.bfloat16
F32 = np.float32

P = 128          # partitions
C = 512          # channels
NH = 8
HS = 64
N = 4096         # pixels (keys)
NQ = 1024        # queries per core
CT = 4           # channel tiles of 128
MT = 32          # m (key) tiles of 128
EPS = 1e-5

dt = mybir.dt
AOT = mybir.AluOpType
ACTF = mybir.ActivationFunctionType
AXT = mybir.AxisListType

_CACHE = {}


def build_program():
    nc = bacc.Bacc("TRN2", target_bir_lowering=False, debug=False, num_devices=8)

    xb_d = nc.dram_tensor("xb", [C, N], dt.bfloat16, kind="ExternalInput")
    xq_d = nc.dram_tensor("xq", [C, NQ], dt.float32, kind="ExternalInput")
    wq_d = nc.dram_tensor("wqT", [C, C], dt.bfloat16, kind="ExternalInput")
    wk_d = nc.dram_tensor("wkT", [C, C], dt.bfloat16, kind="ExternalInput")
    wv_d = nc.dram_tensor("wvT", [C, C], dt.bfloat16, kind="ExternalInput")
    wp_d = nc.dram_tensor("wpT", [C, C], dt.bfloat16, kind="ExternalInput")
    gnw_d = nc.dram_tensor("gnw4", [P, CT], dt.float32, kind="ExternalInput")
    gnb_d = nc.dram_tensor("gnb4", [P, CT], dt.float32, kind="ExternalInput")
    qb_d = nc.dram_tensor("qb4", [P, CT], dt.float32, kind="ExternalInput")
    wvb_d = nc.dram_tensor("wvb4", [P, CT], dt.float32, kind="ExternalInput")
    pb_d = nc.dram_tensor("pb4", [P, CT], dt.float32, kind="ExternalInput")
    inda_d = nc.dram_tensor("inda", [P, 8], dt.float32, kind="ExternalInput")
    indb_d = nc.dram_tensor("indb", [8, P], dt.float32, kind="ExternalInput")
    y_d = nc.dram_tensor("y", [C, NQ], dt.float32, kind="ExternalOutput")

    with tile.TileContext(nc) as tc, ExitStack() as ctx:
        const = ctx.enter_context(tc.tile_pool(name="const", bufs=1))
        wpool = ctx.enter_context(tc.tile_pool(name="wpool", bufs=1))
        wspool = ctx.enter_context(tc.tile_pool(name="wspool", bufs=1))
        xpool = ctx.enter_context(tc.tile_pool(name="xpool", bufs=1))
        stat = ctx.enter_context(tc.tile_pool(name="stat", bufs=1))
        kpool = ctx.enter_context(tc.tile_pool(name="kpool", bufs=1))
        qpool = ctx.enter_context(tc.tile_pool(name="qpool", bufs=1))
        vpool = ctx.enter_context(tc.tile_pool(name="vpool", bufs=1))
        epool = ctx.enter_context(tc.tile_pool(name="epool", bufs=3))
        onpool = ctx.enter_context(tc.tile_pool(name="onpool", bufs=1))
        rpool = ctx.enter_context(tc.tile_pool(name="rpool", bufs=2))
        rsdpool = ctx.enter_context(tc.tile_pool(name="rsdpool", bufs=2))
        outpool = ctx.enter_context(tc.tile_pool(name="outpool", bufs=2))

        # ---------------- constants ----------------
        gnw = const.tile([P, CT], dt.float32)
        nc.sync.dma_start(gnw[:], gnw_d.ap())
        gnb = const.tile([P, CT], dt.float32)
        nc.sync.dma_start(gnb[:], gnb_d.ap())
        qb4 = const.tile([P, CT], dt.float32)
        nc.sync.dma_start(qb4[:], qb_d.ap())
        wvb4 = const.tile([P, CT], dt.float32)
        nc.sync.dma_start(wvb4[:], wvb_d.ap())
        pb4 = const.tile([P, CT], dt.float32)
        nc.sync.dma_start(pb4[:], pb_d.ap())
        inda = const.tile([P, 8], dt.float32)
        nc.sync.dma_start(inda[:], inda_d.ap())
        indb = const.tile([8, P], dt.float32)
        nc.sync.dma_start(indb[:], indb_d.ap())
        ones64 = const.tile([P, HS], dt.bfloat16)
        nc.vector.memset(ones64[:], 1.0)

        # weights: [128, 2048] bf16, col block kt*512..+512 = W^T[kt*128..+128, :]
        wsb = {}
        for nm, d in (("q", wq_d), ("k", wk_d), ("v", wv_d), ("p", wp_d)):
            w = wpool.tile([P, CT * C], dt.bfloat16, tag=f"w_{nm}", name=f"w_{nm}")
            for kt in range(CT):
                nc.sync.dma_start(
                    w[:, kt * C:(kt + 1) * C], d.ap()[kt * P:(kt + 1) * P, :]
                )
            wsb[nm] = w

        # ---------------- phase A: x load + groupnorm stats ----------------
        xt = []
        st = stat.tile([P, 8], dt.float32)
        sq = stat.tile([P, N], dt.bfloat16, tag="sq_scratch")
        for t in range(CT):
            x = xpool.tile([P, N], dt.bfloat16, name=f"xt{t}")
            nc.sync.dma_start(x[:], xb_d.ap()[t * P:(t + 1) * P, :])
            xt.append(x)
            nc.vector.tensor_reduce(st[:, t:t + 1], x[:], axis=AXT.X, op=AOT.add)
            nc.scalar.activation(
                sq[:], x[:], ACTF.Square, accum_out=st[:, 4 + t:5 + t]
            )

        psctx = ExitStack()
        psmall = psctx.enter_context(tc.tile_pool(name="psmall", bufs=2, space="PSUM"))

        gs_ps = psmall.tile([8, 8], dt.float32, tag="small8")
        nc.tensor.matmul(gs_ps[:], lhsT=inda[:], rhs=st[:], start=True, stop=True)
        gstats = stat.tile([8, 8], dt.float32)
        nc.vector.tensor_scalar_mul(gstats[:], gs_ps[:], 1.0 / float(C // 32 * N))
        musq = stat.tile([8, 4], dt.float32)
        nc.vector.tensor_mul(musq[:], gstats[:, 0:4], gstats[:, 0:4])
        var = stat.tile([8, 4], dt.float32)
        nc.vector.tensor_sub(var[:], gstats[:, 4:8], musq[:])
        nc.vector.tensor_scalar_add(var[:], var[:], float(EPS))
        lnv = stat.tile([8, 4], dt.float32)
        nc.scalar.activation(lnv[:], var[:], ACTF.Ln)
        rmu = stat.tile([8, 8], dt.float32)
        nc.scalar.activation(rmu[:, 0:4], lnv[:], ACTF.Exp, scale=-0.5)
        nc.vector.tensor_copy(rmu[:, 4:8], gstats[:, 0:4])
        bc_ps = psmall.tile([P, 8], dt.float32, tag="small8")
        nc.tensor.matmul(bc_ps[:], lhsT=indb[:], rhs=rmu[:], start=True, stop=True)
        ab = stat.tile([P, 8], dt.float32)
        nc.vector.tensor_mul(ab[:, 0:4], bc_ps[:, 0:4], gnw[:])          # a
        mua = stat.tile([P, 4], dt.float32)
        nc.vector.tensor_mul(mua[:], bc_ps[:, 4:8], ab[:, 0:4])          # mu*a
        nc.vector.tensor_sub(ab[:, 4:8], gnb[:], mua[:])                 # b
        bgn = stat.tile([P, 4], dt.bfloat16)
        nc.vector.tensor_copy(bgn[:], ab[:, 4:8])

        # bias matmuls on UNSCALED weights: Wq b, Wv b; then cout = Wp vb + pb
        qbias = stat.tile([P, CT], dt.float32)
        vbias = stat.tile([P, CT], dt.float32)
        for dtile in range(CT):
            pq = psmall.tile([P, 1], dt.float32, tag="small1", name="pq")
            for kt in range(CT):
                nc.tensor.matmul(
                    pq[:],
                    lhsT=wsb["q"][:, kt * C + dtile * P:kt * C + (dtile + 1) * P],
                    rhs=bgn[:, kt:kt + 1],
                    start=(kt == 0), stop=(kt == CT - 1),
                )
            nc.vector.tensor_add(qbias[:, dtile:dtile + 1], pq[:], qb4[:, dtile:dtile + 1])
            pv = psmall.tile([P, 1], dt.float32, tag="small1", name="pv")
            for kt in range(CT):
                nc.tensor.matmul(
                    pv[:],
                    lhsT=wsb["v"][:, kt * C + dtile * P:kt * C + (dtile + 1) * P],
                    rhs=bgn[:, kt:kt + 1],
                    start=(kt == 0), stop=(kt == CT - 1),
                )
            nc.vector.tensor_add(vbias[:, dtile:dtile + 1], pv[:], wvb4[:, dtile:dtile + 1])
        vbias_bf = stat.tile([P, CT], dt.bfloat16)
        nc.vector.tensor_copy(vbias_bf[:], vbias[:])
        cout = stat.tile([P, CT], dt.float32)
        for ct in range(CT):
            pc = psmall.tile([P, 1], dt.float32, tag="small1", name="pc")
            for kt in range(CT):
                nc.tensor.matmul(
                    pc[:],
                    lhsT=wsb["p"][:, kt * C + ct * P:kt * C + (ct + 1) * P],
                    rhs=vbias_bf[:, kt:kt + 1],
                    start=(kt == 0), stop=(kt == CT - 1),
                )
            nc.vector.tensor_add(cout[:, ct:ct + 1], pc[:], pb4[:, ct:ct + 1])

        # scaled weights (groupnorm 'a' folded in): ws = w * a[channel]
        ws = {}
        for nm in ("q", "k", "v"):
            w = wspool.tile([P, CT * C], dt.bfloat16, tag=f"ws_{nm}", name=f"ws_{nm}")
            for kt in range(CT):
                nc.vector.tensor_scalar_mul(
                    w[:, kt * C:(kt + 1) * C],
                    wsb[nm][:, kt * C:(kt + 1) * C],
                    ab[:, kt:kt + 1],
                )
            ws[nm] = w

        # ---------------- phase B: QKV GEMMs ----------------
        psB = psctx.enter_context(tc.tile_pool(name="psB", bufs=4, space="PSUM"))

        # qT[dtile]: [128, 1024]; copy adds q bias
        qT = []
        for dtile in range(CT):
            q = qpool.tile([P, NQ], dt.bfloat16, name=f"qT{dtile}")
            for nch in range(NQ // 512):
                ps = psB.tile([P, 512], dt.float32, name="psb", tag="psb")
                for kt in range(CT):
                    nc.tensor.matmul(
                        ps[:],
                        lhsT=ws["q"][:, kt * C + dtile * P:kt * C + (dtile + 1) * P],
                        rhs=xt[kt][:, nch * 512:(nch + 1) * 512],
                        start=(kt == 0), stop=(kt == CT - 1),
                    )
                nc.vector.tensor_scalar_add(
                    q[:, nch * 512:(nch + 1) * 512], ps[:], qbias[:, dtile:dtile + 1]
                )
            qT.append(q)

        # kT[dtile]: [128, 4096]
        kT = []
        copy_flip = 0
        for dtile in range(CT):
            k = kpool.tile([P, N], dt.bfloat16, name=f"kT{dtile}")
            for nch in range(N // 512):
                ps = psB.tile([P, 512], dt.float32, name="psb", tag="psb")
                for kt in range(CT):
                    nc.tensor.matmul(
                        ps[:],
                        lhsT=ws["k"][:, kt * C + dtile * P:kt * C + (dtile + 1) * P],
                        rhs=xt[kt][:, nch * 512:(nch + 1) * 512],
                        start=(kt == 0), stop=(kt == CT - 1),
                    )
                dst = k[:, nch * 512:(nch + 1) * 512]
                if copy_flip % 2 == 0:
                    nc.scalar.copy(dst, ps[:])
                else:
                    nc.vector.tensor_copy(dst, ps[:])
                copy_flip += 1
            kT.append(k)

        # v[mt]: [128 (m), 512 (d over all heads)]
        vt = []
        for mt in range(MT):
            v = vpool.tile([P, C], dt.bfloat16, name=f"v{mt}")
            ps = psB.tile([P, 512], dt.float32, name="psb", tag="psb")
            for kt in range(CT):
                nc.tensor.matmul(
                    ps[:],
                    lhsT=xt[kt][:, mt * P:(mt + 1) * P],
                    rhs=ws["v"][:, kt * C:(kt + 1) * C],
                    start=(kt == 0), stop=(kt == CT - 1),
                )
            if copy_flip % 2 == 0:
                nc.scalar.copy(v[:], ps[:])
            else:
                nc.vector.tensor_copy(v[:], ps[:])
            copy_flip += 1
            vt.append(v)

        # ---------------- phase C: attention ----------------
        psctx.close()
        spool = ctx.enter_context(tc.tile_pool(name="spool", bufs=2, space="PSUM"))
        opool = ctx.enter_context(tc.tile_pool(name="opool", bufs=1, space="PSUM"))
        obpool = ctx.enter_context(tc.tile_pool(name="obpool", bufs=1, space="PSUM"))

        # steps: (pair, mt, nch) ; QK emission leads PV by one step for pipelining
        steps = [
            (hp, mt, nch)
            for hp in range(NH // 2)
            for mt in range(MT)
            for nch in range(NQ // 512)
        ]

        oa_tiles = {}
        ob_tiles = {}
        s_tiles = {}

        def emit_qk(idx):
            hp, mt, nch = steps[idx]
            s = spool.tile([P, NQ], dt.float32, tag="stile", name=f"s{idx}")
            kk = kT[hp]
            qq = qT[hp]
            nc.tensor.matmul(
                s[:, 0:512],
                lhsT=kk[0:64, mt * P:(mt + 1) * P],
                rhs=qq[0:64, nch * 512:(nch + 1) * 512],
                start=True, stop=True,
            )
            nc.tensor.matmul(
                s[:, 512:1024],
                lhsT=kk[64:128, mt * P:(mt + 1) * P],
                rhs=qq[64:128, nch * 512:(nch + 1) * 512],
                start=True, stop=True,
            )
            s_tiles[idx] = s

        emit_qk(0)
        onorm = []
        for idx, (hp, mt, nch) in enumerate(steps):
            if mt == 0 and nch == 0:
                oa_tiles[hp] = opool.tile([P, NQ], dt.float32, tag="oa", name=f"oa{hp}")
                ob_tiles[hp] = obpool.tile([P, NQ], dt.float32, tag="ob", name=f"ob{hp}")
            if idx + 1 < len(steps):
                emit_qk(idx + 1)
            s = s_tiles.pop(idx)
            e = epool.tile([P, NQ], dt.bfloat16, name=f"e{idx}", tag="e")
            nc.scalar.activation(e[:], s[:], ACTF.Exp, scale=0.125)
            oa = oa_tiles[hp]
            ob = ob_tiles[hp]
            h0, h1 = 2 * hp, 2 * hp + 1
            first = mt == 0
            last = mt == MT - 1
            v = vt[mt]
            ncol = slice(nch * 512, (nch + 1) * 512)
            nc.tensor.matmul(
                oa[0:64, ncol], lhsT=v[:, h0 * HS:(h0 + 1) * HS],
                rhs=e[:, 0:512], start=first, stop=last, skip_group_check=True,
            )
            nc.tensor.matmul(
                oa[64:128, ncol], lhsT=v[:, h1 * HS:(h1 + 1) * HS],
                rhs=e[:, 512:1024], start=first, stop=last, skip_group_check=True,
            )
            nc.tensor.matmul(
                ob[0:64, ncol], lhsT=ones64[:], rhs=e[:, 0:512],
                start=first, stop=last, skip_group_check=True,
            )
            nc.tensor.matmul(
                ob[64:128, ncol], lhsT=ones64[:], rhs=e[:, 512:1024],
                start=first, stop=last, skip_group_check=True,
            )
            if last and nch == NQ // 512 - 1:
                # normalize: o_norm = oA * recip(oB)
                r = rpool.tile([P, NQ], dt.float32, name=f"r{hp}", tag="r")
                nc.vector.reciprocal(r[:], ob[:])
                on = onpool.tile([P, NQ], dt.bfloat16, name=f"on{hp}")
                nc.vector.tensor_mul(on[:], oa[:], r[:])
                onorm.append(on)
                del oa_tiles[hp], ob_tiles[hp]

        # ---------------- phase D: proj + residual + out ----------------
        for ct in range(CT):
            y = spool.tile([P, NQ], dt.float32, tag="stile", name=f"yps{ct}")
            for nch in range(NQ // 512):
                for kt in range(CT):
                    nc.tensor.matmul(
                        y[:, nch * 512:(nch + 1) * 512],
                        lhsT=wsb["p"][:, kt * C + ct * P:kt * C + (ct + 1) * P],
                        rhs=onorm[kt][:, nch * 512:(nch + 1) * 512],
                        start=(kt == 0), stop=(kt == CT - 1),
                    )
            rsd = rsdpool.tile([P, NQ], dt.float32, name=f"rsd{ct}", tag="rsd")
            nc.sync.dma_start(rsd[:], xq_d.ap()[ct * P:(ct + 1) * P, :])
            ot = outpool.tile([P, NQ], dt.float32, name=f"ot{ct}", tag="ot")
            # (y + cout) + resid in one DVE pass
            nc.vector.scalar_tensor_tensor(
                ot[:], y[:], cout[:, ct:ct + 1], rsd[:], AOT.add, AOT.add
            )
            nc.sync.dma_start(y_d.ap()[ct * P:(ct + 1) * P, :], ot[:])

    nc.compile()
    return nc


def make_in_maps(inputs):
    x = np.asarray(inputs["x"], dtype=np.float32).reshape(2, C, N)
    gn_w = np.asarray(inputs["gn_w"], dtype=np.float32)
    gn_b = np.asarray(inputs["gn_b"], dtype=np.float32)

    def t4(v):
        return np.ascontiguousarray(np.asarray(v, np.float32).reshape(CT, P).T)

    wqT = np.ascontiguousarray(np.asarray(inputs["wq_w"], np.float32).T).astype(BF16)
    wkT = np.ascontiguousarray(np.asarray(inputs["wk_w"], np.float32).T).astype(BF16)
    wvT = np.ascontiguousarray(np.asarray(inputs["wv_w"], np.float32).T).astype(BF16)
    wpT = np.ascontiguousarray(np.asarray(inputs["proj_w"], np.float32).T).astype(BF16)
    gnw4 = t4(gn_w)
    gnb4 = t4(gn_b)
    qb4 = t4(inputs["wq_b"])
    wvb4 = t4(inputs["wv_b"])
    pb4 = t4(inputs["proj_b"])
    inda = np.zeros((P, 8), np.float32)
    for p in range(P):
        inda[p, p // 16] = 1.0
    indb = np.ascontiguousarray(inda.T)

    shared = dict(
        wqT=wqT, wkT=wkT, wvT=wvT, wpT=wpT,
        gnw4=gnw4, gnb4=gnb4, qb4=qb4, wvb4=wvb4, pb4=pb4,
        inda=inda, indb=indb,
    )
    in_maps = []
    for core in range(8):
        b, r = core // 4, core % 4
        nq0 = r * NQ
        rolled = np.roll(x[b], -nq0, axis=1)
        m = dict(shared)
        m["xb"] = rolled.astype(BF16)
        m["xq"] = np.ascontiguousarray(x[b][:, nq0:nq0 + NQ])
        in_maps.append(m)
    return in_maps


def assemble(results):
    out = np.empty((2, C, N), np.float32)
    for core in range(8):
        b, r = core // 4, core % 4
        out[b][:, r * NQ:(r + 1) * NQ] = results[core]["y"]
    return out.reshape(2, C, 64, 64)


def get_program():
    if "nc" not in _CACHE:
        _CACHE["nc"] = build_program()
    return _CACHE["nc"]


def kernel(**inputs):
    nc = get_program()
    in_maps = make_in_maps(inputs)
    res = run_bass_kernel_spmd(nc, in_maps, core_ids=list(range(8)))
    return assemble(res.results)


# revision 11
# speedup vs baseline: 1.0304x; 1.0304x over previous
"""Trainium2 Bass kernel for nn_AttentionBlock (GroupNorm + 8-head self-attention + proj + residual).

Full inputs in, full output out. Sharding: 8 cores = 2 batches x 4-way split of
the 4096 query pixels. Each core runs an identical SPMD program on per-core
input data:
  - x for its batch, pixel axis rotated so the core's 1024 query rows sit at
    columns 0..1023 (attention and groupnorm are permutation-invariant over
    keys/pixels, so rotation is exact),
  - shared (host-transposed, bf16) weight matrices.

Device-side math (exact up to fp rounding):
  groupnorm:  h = a (.) x + b   with per-channel a = gn_w * rstd(group),
              b = gn_b - mu(group) * a, computed on-device from bf16 x.
  The affine is folded into the QKV weights on the device:
              q = (Wq diag(a)) x + (Wq b + wq_b),
              k = (Wk diag(a)) x        (k's constant shifts every score row
                                         by a per-query constant -> cancels in
                                         softmax, dropped),
              v = (Wv diag(a)) x        (v's constant vb = Wv b + wv_b rides
                                         through softmax unchanged; it is
                                         applied at the end via
                                         cout = proj_w vb + proj_b).
  attention:  S^T tiles (keys on partitions) = k_tile^T-slices x q; exp via
              ScalarE with the 1/8 scale fused; denominators from a ones-column
              matmul (PV col-packed per head pair); o_norm = oA * recip(oB).
  out:        y^T = proj_w o_norm^T + cout + residual.
"""

import numpy as np
import ml_dtypes
from contextlib import ExitStack

import concourse.bacc as bacc
import concourse.tile as tile
import concourse.mybir as mybir
from concourse.bass_utils import run_bass_kernel_spmd

BF16 = ml_dtypes.bfloat16
F32 = np.float32

P = 128          # partitions
C = 512          # channels
NH = 8
HS = 64
N = 4096         # pixels (keys)
NQ = 1024        # queries per core
CT = 4           # channel tiles of 128
MT = 32          # m (key) tiles of 128
EPS = 1e-5
SCH_A16 = float(2.0 ** 7 / np.log(2.0))   # int16 Schraudolph exp
SCH_B16 = 16250.4062
ACT_FRAC_NUM = 11   # ACT handles 11/20 of exp tiles, DVE the rest
ACT_FRAC_DEN = 20

dt = mybir.dt
AOT = mybir.AluOpType
ACTF = mybir.ActivationFunctionType
AXT = mybir.AxisListType

_CACHE = {}


def build_program():
    nc = bacc.Bacc("TRN2", target_bir_lowering=False, debug=False, num_devices=8)

    xb_d = nc.dram_tensor("xb", [C, N], dt.bfloat16, kind="ExternalInput")
    xq_d = nc.dram_tensor("xq", [C, NQ], dt.float32, kind="ExternalInput")
    wq_d = nc.dram_tensor("wqT", [C, C], dt.bfloat16, kind="ExternalInput")
    wk_d = nc.dram_tensor("wkT", [C, C], dt.bfloat16, kind="ExternalInput")
    wv_d = nc.dram_tensor("wvT", [C, C], dt.bfloat16, kind="ExternalInput")
    wp_d = nc.dram_tensor("wpT", [C, C], dt.bfloat16, kind="ExternalInput")
    gnw_d = nc.dram_tensor("gnw4", [P, CT], dt.float32, kind="ExternalInput")
    gnb_d = nc.dram_tensor("gnb4", [P, CT], dt.float32, kind="ExternalInput")
    qb_d = nc.dram_tensor("qb4", [P, CT], dt.float32, kind="ExternalInput")
    wvb_d = nc.dram_tensor("wvb4", [P, CT], dt.float32, kind="ExternalInput")
    pb_d = nc.dram_tensor("pb4", [P, CT], dt.float32, kind="ExternalInput")
    inda_d = nc.dram_tensor("inda", [P, 8], dt.float32, kind="ExternalInput")
    indb_d = nc.dram_tensor("indb", [8, P], dt.float32, kind="ExternalInput")
    y_d = nc.dram_tensor("y", [C, NQ], dt.float32, kind="ExternalOutput")

    with tile.TileContext(nc) as tc, ExitStack() as ctx:
        const = ctx.enter_context(tc.tile_pool(name="const", bufs=1))
        wpool = ctx.enter_context(tc.tile_pool(name="wpool", bufs=1))
        wspool = ctx.enter_context(tc.tile_pool(name="wspool", bufs=1))
        xpool = ctx.enter_context(tc.tile_pool(name="xpool", bufs=1))
        stat = ctx.enter_context(tc.tile_pool(name="stat", bufs=1))
        kpool = ctx.enter_context(tc.tile_pool(name="kpool", bufs=1))
        qpool = ctx.enter_context(tc.tile_pool(name="qpool", bufs=1))
        vpool = ctx.enter_context(tc.tile_pool(name="vpool", bufs=1))
        epool = ctx.enter_context(tc.tile_pool(name="epool", bufs=3))
        onpool = ctx.enter_context(tc.tile_pool(name="onpool", bufs=1))
        rpool = ctx.enter_context(tc.tile_pool(name="rpool", bufs=2))
        rsdpool = ctx.enter_context(tc.tile_pool(name="rsdpool", bufs=2))
        outpool = ctx.enter_context(tc.tile_pool(name="outpool", bufs=2))

        # ---------------- constants ----------------
        gnw = const.tile([P, CT], dt.float32)
        nc.sync.dma_start(gnw[:], gnw_d.ap())
        gnb = const.tile([P, CT], dt.float32)
        nc.sync.dma_start(gnb[:], gnb_d.ap())
        qb4 = const.tile([P, CT], dt.float32)
        nc.sync.dma_start(qb4[:], qb_d.ap())
        wvb4 = const.tile([P, CT], dt.float32)
        nc.sync.dma_start(wvb4[:], wvb_d.ap())
        pb4 = const.tile([P, CT], dt.float32)
        nc.sync.dma_start(pb4[:], pb_d.ap())
        inda = const.tile([P, 8], dt.float32)
        nc.sync.dma_start(inda[:], inda_d.ap())
        indb = const.tile([8, P], dt.float32)
        nc.sync.dma_start(indb[:], indb_d.ap())
        ones64 = const.tile([P, HS], dt.bfloat16)
        nc.vector.memset(ones64[:], 1.0)

        # weights: [128, 2048] bf16, col block kt*512..+512 = W^T[kt*128..+128, :]
        wsb = {}
        for nm, d in (("q", wq_d), ("k", wk_d), ("v", wv_d), ("p", wp_d)):
            w = wpool.tile([P, CT * C], dt.bfloat16, tag=f"w_{nm}", name=f"w_{nm}")
            for kt in range(CT):
                nc.sync.dma_start(
                    w[:, kt * C:(kt + 1) * C], d.ap()[kt * P:(kt + 1) * P, :]
                )
            wsb[nm] = w

        # ---------------- phase A: x load + groupnorm stats ----------------
        xt = []
        st = stat.tile([P, 8], dt.float32)
        sq = stat.tile([P, N], dt.bfloat16, tag="sq_scratch")
        for t in range(CT):
            x = xpool.tile([P, N], dt.bfloat16, name=f"xt{t}")
            nc.sync.dma_start(x[:], xb_d.ap()[t * P:(t + 1) * P, :])
            xt.append(x)
            nc.vector.tensor_reduce(st[:, t:t + 1], x[:], axis=AXT.X, op=AOT.add)
            nc.scalar.activation(
                sq[:], x[:], ACTF.Square, accum_out=st[:, 4 + t:5 + t]
            )

        psctx = ExitStack()
        psmall = psctx.enter_context(tc.tile_pool(name="psmall", bufs=2, space="PSUM"))

        gs_ps = psmall.tile([8, 8], dt.float32, tag="small8")
        nc.tensor.matmul(gs_ps[:], lhsT=inda[:], rhs=st[:], start=True, stop=True)
        gstats = stat.tile([8, 8], dt.float32)
        nc.vector.tensor_scalar_mul(gstats[:], gs_ps[:], 1.0 / float(C // 32 * N))
        musq = stat.tile([8, 4], dt.float32)
        nc.vector.tensor_mul(musq[:], gstats[:, 0:4], gstats[:, 0:4])
        var = stat.tile([8, 4], dt.float32)
        nc.vector.tensor_sub(var[:], gstats[:, 4:8], musq[:])
        nc.vector.tensor_scalar_add(var[:], var[:], float(EPS))
        lnv = stat.tile([8, 4], dt.float32)
        nc.scalar.activation(lnv[:], var[:], ACTF.Ln)
        rmu = stat.tile([8, 8], dt.float32)
        nc.scalar.activation(rmu[:, 0:4], lnv[:], ACTF.Exp, scale=-0.5)
        nc.vector.tensor_copy(rmu[:, 4:8], gstats[:, 0:4])
        bc_ps = psmall.tile([P, 8], dt.float32, tag="small8")
        nc.tensor.matmul(bc_ps[:], lhsT=indb[:], rhs=rmu[:], start=True, stop=True)
        ab = stat.tile([P, 8], dt.float32)
        nc.vector.tensor_mul(ab[:, 0:4], bc_ps[:, 0:4], gnw[:])          # a
        mua = stat.tile([P, 4], dt.float32)
        nc.vector.tensor_mul(mua[:], bc_ps[:, 4:8], ab[:, 0:4])          # mu*a
        nc.vector.tensor_sub(ab[:, 4:8], gnb[:], mua[:])                 # b
        bgn = stat.tile([P, 4], dt.bfloat16)
        nc.vector.tensor_copy(bgn[:], ab[:, 4:8])

        # bias matmuls on UNSCALED weights: Wq b, Wv b; then cout = Wp vb + pb
        qbias = stat.tile([P, CT], dt.float32)
        vbias = stat.tile([P, CT], dt.float32)
        for dtile in range(CT):
            pq = psmall.tile([P, 1], dt.float32, tag="small1", name="pq")
            for kt in range(CT):
                nc.tensor.matmul(
                    pq[:],
                    lhsT=wsb["q"][:, kt * C + dtile * P:kt * C + (dtile + 1) * P],
                    rhs=bgn[:, kt:kt + 1],
                    start=(kt == 0), stop=(kt == CT - 1),
                )
            nc.vector.tensor_add(qbias[:, dtile:dtile + 1], pq[:], qb4[:, dtile:dtile + 1])
            pv = psmall.tile([P, 1], dt.float32, tag="small1", name="pv")
            for kt in range(CT):
                nc.tensor.matmul(
                    pv[:],
                    lhsT=wsb["v"][:, kt * C + dtile * P:kt * C + (dtile + 1) * P],
                    rhs=bgn[:, kt:kt + 1],
                    start=(kt == 0), stop=(kt == CT - 1),
                )
            nc.vector.tensor_add(vbias[:, dtile:dtile + 1], pv[:], wvb4[:, dtile:dtile + 1])
        vbias_bf = stat.tile([P, CT], dt.bfloat16)
        nc.vector.tensor_copy(vbias_bf[:], vbias[:])
        cout = stat.tile([P, CT], dt.float32)
        for ct in range(CT):
            pc = psmall.tile([P, 1], dt.float32, tag="small1", name="pc")
            for kt in range(CT):
                nc.tensor.matmul(
                    pc[:],
                    lhsT=wsb["p"][:, kt * C + ct * P:kt * C + (ct + 1) * P],
                    rhs=vbias_bf[:, kt:kt + 1],
                    start=(kt == 0), stop=(kt == CT - 1),
                )
            nc.vector.tensor_add(cout[:, ct:ct + 1], pc[:], pb4[:, ct:ct + 1])

        # scaled weights (groupnorm 'a' folded in): ws = w * a[channel]
        ws = {}
        for nm in ("q", "k", "v"):
            w = wspool.tile([P, CT * C], dt.bfloat16, tag=f"ws_{nm}", name=f"ws_{nm}")
            for kt in range(CT):
                nc.vector.tensor_scalar_mul(
                    w[:, kt * C:(kt + 1) * C],
                    wsb[nm][:, kt * C:(kt + 1) * C],
                    ab[:, kt:kt + 1],
                )
            ws[nm] = w

        # ---------------- phase B: QKV GEMMs ----------------
        psB = psctx.enter_context(tc.tile_pool(name="psB", bufs=4, space="PSUM"))

        # qT[dtile]: [128, 1024]; copy adds q bias
        qT = []
        for dtile in range(CT):
            q = qpool.tile([P, NQ], dt.bfloat16, name=f"qT{dtile}")
            for nch in range(NQ // 512):
                ps = psB.tile([P, 512], dt.float32, name="psb", tag="psb")
                for kt in range(CT):
                    nc.tensor.matmul(
                        ps[:],
                        lhsT=ws["q"][:, kt * C + dtile * P:kt * C + (dtile + 1) * P],
                        rhs=xt[kt][:, nch * 512:(nch + 1) * 512],
                        start=(kt == 0), stop=(kt == CT - 1),
                    )
                nc.vector.tensor_scalar_add(
                    q[:, nch * 512:(nch + 1) * 512], ps[:], qbias[:, dtile:dtile + 1]
                )
            qT.append(q)

        # kT[dtile]: [128, 4096]
        kT = []
        copy_flip = 0
        for dtile in range(CT):
            k = kpool.tile([P, N], dt.bfloat16, name=f"kT{dtile}")
            for nch in range(N // 512):
                ps = psB.tile([P, 512], dt.float32, name="psb", tag="psb")
                for kt in range(CT):
                    nc.tensor.matmul(
                        ps[:],
                        lhsT=ws["k"][:, kt * C + dtile * P:kt * C + (dtile + 1) * P],
                        rhs=xt[kt][:, nch * 512:(nch + 1) * 512],
                        start=(kt == 0), stop=(kt == CT - 1),
                    )
                dst = k[:, nch * 512:(nch + 1) * 512]
                if copy_flip % 2 == 0:
                    nc.scalar.copy(dst, ps[:])
                else:
                    nc.vector.tensor_copy(dst, ps[:])
                copy_flip += 1
            kT.append(k)

        # v[mt]: [128 (m), 512 (d over all heads)]
        vt = []
        for mt in range(MT):
            v = vpool.tile([P, C], dt.bfloat16, name=f"v{mt}")
            ps = psB.tile([P, 512], dt.float32, name="psb", tag="psb")
            for kt in range(CT):
                nc.tensor.matmul(
                    ps[:],
                    lhsT=xt[kt][:, mt * P:(mt + 1) * P],
                    rhs=ws["v"][:, kt * C:(kt + 1) * C],
                    start=(kt == 0), stop=(kt == CT - 1),
                )
            if copy_flip % 2 == 0:
                nc.scalar.copy(v[:], ps[:])
            else:
                nc.vector.tensor_copy(v[:], ps[:])
            copy_flip += 1
            vt.append(v)

        # ---------------- phase C: attention ----------------
        psctx.close()
        spool = ctx.enter_context(tc.tile_pool(name="spool", bufs=2, space="PSUM"))
        opool = ctx.enter_context(tc.tile_pool(name="opool", bufs=1, space="PSUM"))
        obpool = ctx.enter_context(tc.tile_pool(name="obpool", bufs=1, space="PSUM"))

        # steps: (pair, mt, nch) ; QK emission leads PV by one step for pipelining
        steps = [
            (hp, mt, nch)
            for hp in range(NH // 2)
            for mt in range(MT)
            for nch in range(NQ // 512)
        ]

        oa_tiles = {}
        ob_tiles = {}
        s_tiles = {}

        def emit_qk(idx):
            hp, mt, nch = steps[idx]
            s = spool.tile([P, NQ], dt.float32, tag="stile", name=f"s{idx}")
            kk = kT[hp]
            qq = qT[hp]
            nc.tensor.matmul(
                s[:, 0:512],
                lhsT=kk[0:64, mt * P:(mt + 1) * P],
                rhs=qq[0:64, nch * 512:(nch + 1) * 512],
                start=True, stop=True,
            )
            nc.tensor.matmul(
                s[:, 512:1024],
                lhsT=kk[64:128, mt * P:(mt + 1) * P],
                rhs=qq[64:128, nch * 512:(nch + 1) * 512],
                start=True, stop=True,
            )
            s_tiles[idx] = s

        emit_qk(0)
        onorm = []
        for idx, (hp, mt, nch) in enumerate(steps):
            if mt == 0 and nch == 0:
                oa_tiles[hp] = opool.tile([P, NQ], dt.float32, tag="oa", name=f"oa{hp}")
                ob_tiles[hp] = obpool.tile([P, NQ], dt.float32, tag="ob", name=f"ob{hp}")
            if idx + 1 < len(steps):
                emit_qk(idx + 1)
            s = s_tiles.pop(idx)
            if (idx * ACT_FRAC_NUM) % ACT_FRAC_DEN < ACT_FRAC_NUM:
                e = epool.tile([P, NQ], dt.bfloat16, name=f"e{idx}", tag="e")
                nc.scalar.activation(e[:], s[:], ACTF.Exp, scale=0.125)
                ea = e[:]
            else:
                ei = epool.tile([P, NQ], dt.int16, name=f"e{idx}", tag="e")
                nc.vector.tensor_scalar(
                    ei[:], s[:], SCH_A16 * 0.125, SCH_B16, AOT.mult, AOT.add
                )
                ea = ei[:].bitcast(dt.bfloat16)
            oa = oa_tiles[hp]
            ob = ob_tiles[hp]
            h0, h1 = 2 * hp, 2 * hp + 1
            first = mt == 0
            last = mt == MT - 1
            v = vt[mt]
            ncol = slice(nch * 512, (nch + 1) * 512)
            nc.tensor.matmul(
                oa[0:64, ncol], lhsT=v[:, h0 * HS:(h0 + 1) * HS],
                rhs=ea[:, 0:512], start=first, stop=last, skip_group_check=True,
            )
            nc.tensor.matmul(
                oa[64:128, ncol], lhsT=v[:, h1 * HS:(h1 + 1) * HS],
                rhs=ea[:, 512:1024], start=first, stop=last, skip_group_check=True,
            )
            nc.tensor.matmul(
                ob[0:64, ncol], lhsT=ones64[:], rhs=ea[:, 0:512],
                start=first, stop=last, skip_group_check=True,
            )
            nc.tensor.matmul(
                ob[64:128, ncol], lhsT=ones64[:], rhs=ea[:, 512:1024],
                start=first, stop=last, skip_group_check=True,
            )
            if last and nch == NQ // 512 - 1:
                # normalize: o_norm = oA * recip_fast(oB); ~2.5us psum hold
                r = rpool.tile([P, NQ], dt.float32, name=f"r{hp}", tag="r")
                nc.vector.reciprocal_approx_fast(r[:], ob[:])
                on = onpool.tile([P, NQ], dt.bfloat16, name=f"on{hp}")
                nc.vector.tensor_mul(on[:], oa[:], r[:])
                onorm.append(on)
                del oa_tiles[hp], ob_tiles[hp]

        # ---------------- phase D: proj + residual + out ----------------
        for ct in range(CT):
            y = spool.tile([P, NQ], dt.float32, tag="stile", name=f"yps{ct}")
            for nch in range(NQ // 512):
                for kt in range(CT):
                    nc.tensor.matmul(
                        y[:, nch * 512:(nch + 1) * 512],
                        lhsT=wsb["p"][:, kt * C + ct * P:kt * C + (ct + 1) * P],
                        rhs=onorm[kt][:, nch * 512:(nch + 1) * 512],
                        start=(kt == 0), stop=(kt == CT - 1),
                    )
            rsd = rsdpool.tile([P, NQ], dt.float32, name=f"rsd{ct}", tag="rsd")
            nc.sync.dma_start(rsd[:], xq_d.ap()[ct * P:(ct + 1) * P, :])
            ot = outpool.tile([P, NQ], dt.float32, name=f"ot{ct}", tag="ot")
            # (y + cout) + resid in one DVE pass
            nc.vector.scalar_tensor_tensor(
                ot[:], y[:], cout[:, ct:ct + 1], rsd[:], AOT.add, AOT.add
            )
            nc.sync.dma_start(y_d.ap()[ct * P:(ct + 1) * P, :], ot[:])

    nc.compile()
    return nc


def make_in_maps(inputs):
    x = np.asarray(inputs["x"], dtype=np.float32).reshape(2, C, N)
    gn_w = np.asarray(inputs["gn_w"], dtype=np.float32)
    gn_b = np.asarray(inputs["gn_b"], dtype=np.float32)

    def t4(v):
        return np.ascontiguousarray(np.asarray(v, np.float32).reshape(CT, P).T)

    wqT = np.ascontiguousarray(np.asarray(inputs["wq_w"], np.float32).T).astype(BF16)
    wkT = np.ascontiguousarray(np.asarray(inputs["wk_w"], np.float32).T).astype(BF16)
    wvT = np.ascontiguousarray(np.asarray(inputs["wv_w"], np.float32).T).astype(BF16)
    wpT = np.ascontiguousarray(np.asarray(inputs["proj_w"], np.float32).T).astype(BF16)
    gnw4 = t4(gn_w)
    gnb4 = t4(gn_b)
    qb4 = t4(inputs["wq_b"])
    wvb4 = t4(inputs["wv_b"])
    pb4 = t4(inputs["proj_b"])
    inda = np.zeros((P, 8), np.float32)
    for p in range(P):
        inda[p, p // 16] = 1.0
    indb = np.ascontiguousarray(inda.T)

    shared = dict(
        wqT=wqT, wkT=wkT, wvT=wvT, wpT=wpT,
        gnw4=gnw4, gnb4=gnb4, qb4=qb4, wvb4=wvb4, pb4=pb4,
        inda=inda, indb=indb,
    )
    in_maps = []
    for core in range(8):
        b, r = core // 4, core % 4
        nq0 = r * NQ
        rolled = np.roll(x[b], -nq0, axis=1)
        m = dict(shared)
        m["xb"] = rolled.astype(BF16)
        m["xq"] = np.ascontiguousarray(x[b][:, nq0:nq0 + NQ])
        in_maps.append(m)
    return in_maps


def assemble(results):
    out = np.empty((2, C, N), np.float32)
    for core in range(8):
        b, r = core // 4, core % 4
        out[b][:, r * NQ:(r + 1) * NQ] = results[core]["y"]
    return out.reshape(2, C, 64, 64)


def get_program():
    if "nc" not in _CACHE:
        _CACHE["nc"] = build_program()
    return _CACHE["nc"]


def kernel(**inputs):
    nc = get_program()
    in_maps = make_in_maps(inputs)
    res = run_bass_kernel_spmd(nc, in_maps, core_ids=list(range(8)))
    return assemble(res.results)


# revision 14
# speedup vs baseline: 1.2114x; 1.1756x over previous
"""Trainium2 Bass kernel for nn_AttentionBlock (GroupNorm + 8-head self-attention + proj + residual).

Full inputs in, full output out. Sharding: 8 cores = 2 batches x 4-way split of
the 4096 query pixels. Each core runs an identical SPMD program on per-core
input data:
  - x for its batch, pixel axis rotated so the core's 1024 query rows sit at
    columns 0..1023 (attention and groupnorm are permutation-invariant over
    keys/pixels, so rotation is exact),
  - shared (host-transposed, bf16) weight matrices.

Device-side math (exact up to fp rounding):
  groupnorm:  h = a (.) x + b   with per-channel a = gn_w * rstd(group),
              b = gn_b - mu(group) * a, computed on-device from bf16 x.
  The affine is folded into the QKV weights on the device:
              q = (Wq diag(a)) x + (Wq b + wq_b),
              k = (Wk diag(a)) x        (k's constant shifts every score row
                                         by a per-query constant -> cancels in
                                         softmax, dropped),
              v = (Wv diag(a)) x        (v's constant vb = Wv b + wv_b rides
                                         through softmax unchanged; it is
                                         applied at the end via
                                         cout = proj_w vb + proj_b).
  attention:  S^T tiles (keys on partitions) = k_tile^T-slices x q; exp via
              ScalarE with the 1/8 scale fused; denominators from a ones-column
              matmul (PV col-packed per head pair); o_norm = oA * recip(oB).
  out:        y^T = proj_w o_norm^T + cout + residual.
"""

import numpy as np
import ml_dtypes
from contextlib import ExitStack

import concourse.bacc as bacc
import concourse.tile as tile
import concourse.mybir as mybir
from concourse.bass_utils import run_bass_kernel_spmd

BF16 = ml_dtypes.bfloat16
F32 = np.float32

P = 128          # partitions
C = 512          # channels
NH = 8
HS = 64
N = 4096         # pixels (keys)
NQ = 1024        # queries per core
CT = 4           # channel tiles of 128
MT = 32          # m (key) tiles of 128
EPS = 1e-5
SCH_A16 = float(2.0 ** 7 / np.log(2.0))   # int16 Schraudolph exp
SCH_B16 = 16250.4062
ACT_FRAC_NUM = 11   # ACT handles 11/20 of exp tiles, DVE the rest
ACT_FRAC_DEN = 20

dt = mybir.dt
AOT = mybir.AluOpType
ACTF = mybir.ActivationFunctionType
AXT = mybir.AxisListType

_CACHE = {}


def build_program():
    nc = bacc.Bacc("TRN2", target_bir_lowering=False, debug=False, num_devices=8)

    xb_d = nc.dram_tensor("xb", [C, N], dt.bfloat16, kind="ExternalInput")
    xq_d = nc.dram_tensor("xq", [C, NQ], dt.float32, kind="ExternalInput")
    wq_d = nc.dram_tensor("wqT", [C, C], dt.bfloat16, kind="ExternalInput")
    wk_d = nc.dram_tensor("wkT", [C, C], dt.bfloat16, kind="ExternalInput")
    wv_d = nc.dram_tensor("wvT", [C, C], dt.bfloat16, kind="ExternalInput")
    wp_d = nc.dram_tensor("wpT", [C, C], dt.bfloat16, kind="ExternalInput")
    gnw_d = nc.dram_tensor("gnw4", [P, CT], dt.float32, kind="ExternalInput")
    gnb_d = nc.dram_tensor("gnb4", [P, CT], dt.float32, kind="ExternalInput")
    qb_d = nc.dram_tensor("qb4", [P, CT], dt.float32, kind="ExternalInput")
    wvb_d = nc.dram_tensor("wvb4", [P, CT], dt.float32, kind="ExternalInput")
    pb_d = nc.dram_tensor("pb4", [P, CT], dt.float32, kind="ExternalInput")
    inda_d = nc.dram_tensor("inda", [P, 8], dt.float32, kind="ExternalInput")
    indb_d = nc.dram_tensor("indb", [8, P], dt.float32, kind="ExternalInput")
    y_d = nc.dram_tensor("y", [C, NQ], dt.float32, kind="ExternalOutput")

    with tile.TileContext(nc) as tc, ExitStack() as ctx:
        const = ctx.enter_context(tc.tile_pool(name="const", bufs=1))
        wpool = ctx.enter_context(tc.tile_pool(name="wpool", bufs=1))
        wspool = ctx.enter_context(tc.tile_pool(name="wspool", bufs=1))
        xpool = ctx.enter_context(tc.tile_pool(name="xpool", bufs=1))
        stat = ctx.enter_context(tc.tile_pool(name="stat", bufs=1))
        kpool = ctx.enter_context(tc.tile_pool(name="kpool", bufs=1))
        qpool = ctx.enter_context(tc.tile_pool(name="qpool", bufs=1))
        vpool = ctx.enter_context(tc.tile_pool(name="vpool", bufs=1))
        epool = ctx.enter_context(tc.tile_pool(name="epool", bufs=3))
        onpool = ctx.enter_context(tc.tile_pool(name="onpool", bufs=1))
        rpool = ctx.enter_context(tc.tile_pool(name="rpool", bufs=2))
        rsdpool = ctx.enter_context(tc.tile_pool(name="rsdpool", bufs=2))
        outpool = ctx.enter_context(tc.tile_pool(name="outpool", bufs=2))

        # ---------------- constants ----------------
        gnw = const.tile([P, CT], dt.float32)
        nc.sync.dma_start(gnw[:], gnw_d.ap())
        gnb = const.tile([P, CT], dt.float32)
        nc.sync.dma_start(gnb[:], gnb_d.ap())
        qb4 = const.tile([P, CT], dt.float32)
        nc.sync.dma_start(qb4[:], qb_d.ap())
        wvb4 = const.tile([P, CT], dt.float32)
        nc.sync.dma_start(wvb4[:], wvb_d.ap())
        pb4 = const.tile([P, CT], dt.float32)
        nc.sync.dma_start(pb4[:], pb_d.ap())
        inda = const.tile([P, 8], dt.float32)
        nc.sync.dma_start(inda[:], inda_d.ap())
        indb = const.tile([8, P], dt.float32)
        nc.sync.dma_start(indb[:], indb_d.ap())
        ones64 = const.tile([P, HS], dt.bfloat16)
        nc.vector.memset(ones64[:], 1.0)

        # weights: [128, 2048] bf16, col block kt*512..+512 = W^T[kt*128..+128, :]
        wsb = {}
        for nm, d in (("q", wq_d), ("k", wk_d), ("v", wv_d), ("p", wp_d)):
            w = wpool.tile([P, CT * C], dt.bfloat16, tag=f"w_{nm}", name=f"w_{nm}")
            for kt in range(CT):
                nc.sync.dma_start(
                    w[:, kt * C:(kt + 1) * C], d.ap()[kt * P:(kt + 1) * P, :]
                )
            wsb[nm] = w

        # ---------------- phase A: x load + groupnorm stats ----------------
        xt = []
        st = stat.tile([P, 8], dt.float32)
        sq = stat.tile([P, N], dt.bfloat16, tag="sq_scratch")
        for t in range(CT):
            x = xpool.tile([P, N], dt.bfloat16, name=f"xt{t}")
            nc.sync.dma_start(x[:], xb_d.ap()[t * P:(t + 1) * P, :])
            xt.append(x)
            nc.vector.tensor_reduce(st[:, t:t + 1], x[:], axis=AXT.X, op=AOT.add)
            nc.scalar.activation(
                sq[:], x[:], ACTF.Square, accum_out=st[:, 4 + t:5 + t]
            )

        psctx = ExitStack()
        psmall = psctx.enter_context(tc.tile_pool(name="psmall", bufs=2, space="PSUM"))

        gs_ps = psmall.tile([8, 8], dt.float32, tag="small8")
        nc.tensor.matmul(gs_ps[:], lhsT=inda[:], rhs=st[:], start=True, stop=True)
        gstats = stat.tile([8, 8], dt.float32)
        nc.vector.tensor_scalar_mul(gstats[:], gs_ps[:], 1.0 / float(C // 32 * N))
        musq = stat.tile([8, 4], dt.float32)
        nc.vector.tensor_mul(musq[:], gstats[:, 0:4], gstats[:, 0:4])
        var = stat.tile([8, 4], dt.float32)
        nc.vector.tensor_sub(var[:], gstats[:, 4:8], musq[:])
        nc.vector.tensor_scalar_add(var[:], var[:], float(EPS))
        lnv = stat.tile([8, 4], dt.float32)
        nc.scalar.activation(lnv[:], var[:], ACTF.Ln)
        rmu = stat.tile([8, 8], dt.float32)
        nc.scalar.activation(rmu[:, 0:4], lnv[:], ACTF.Exp, scale=-0.5)
        nc.vector.tensor_copy(rmu[:, 4:8], gstats[:, 0:4])
        bc_ps = psmall.tile([P, 8], dt.float32, tag="small8")
        nc.tensor.matmul(bc_ps[:], lhsT=indb[:], rhs=rmu[:], start=True, stop=True)
        ab = stat.tile([P, 8], dt.float32)
        nc.vector.tensor_mul(ab[:, 0:4], bc_ps[:, 0:4], gnw[:])          # a
        mua = stat.tile([P, 4], dt.float32)
        nc.vector.tensor_mul(mua[:], bc_ps[:, 4:8], ab[:, 0:4])          # mu*a
        nc.vector.tensor_sub(ab[:, 4:8], gnb[:], mua[:])                 # b
        bgn = stat.tile([P, 4], dt.bfloat16)
        nc.vector.tensor_copy(bgn[:], ab[:, 4:8])

        # bias matmuls on UNSCALED weights: Wq b, Wv b; then cout = Wp vb + pb
        qbias = stat.tile([P, CT], dt.float32)
        vbias = stat.tile([P, CT], dt.float32)
        for dtile in range(CT):
            pq = psmall.tile([P, 1], dt.float32, tag="small1", name="pq")
            for kt in range(CT):
                nc.tensor.matmul(
                    pq[:],
                    lhsT=wsb["q"][:, kt * C + dtile * P:kt * C + (dtile + 1) * P],
                    rhs=bgn[:, kt:kt + 1],
                    start=(kt == 0), stop=(kt == CT - 1),
                )
            nc.vector.tensor_add(qbias[:, dtile:dtile + 1], pq[:], qb4[:, dtile:dtile + 1])
            pv = psmall.tile([P, 1], dt.float32, tag="small1", name="pv")
            for kt in range(CT):
                nc.tensor.matmul(
                    pv[:],
                    lhsT=wsb["v"][:, kt * C + dtile * P:kt * C + (dtile + 1) * P],
                    rhs=bgn[:, kt:kt + 1],
                    start=(kt == 0), stop=(kt == CT - 1),
                )
            nc.vector.tensor_add(vbias[:, dtile:dtile + 1], pv[:], wvb4[:, dtile:dtile + 1])
        vbias_bf = stat.tile([P, CT], dt.bfloat16)
        nc.vector.tensor_copy(vbias_bf[:], vbias[:])
        cout = stat.tile([P, CT], dt.float32)
        for ct in range(CT):
            pc = psmall.tile([P, 1], dt.float32, tag="small1", name="pc")
            for kt in range(CT):
                nc.tensor.matmul(
                    pc[:],
                    lhsT=wsb["p"][:, kt * C + ct * P:kt * C + (ct + 1) * P],
                    rhs=vbias_bf[:, kt:kt + 1],
                    start=(kt == 0), stop=(kt == CT - 1),
                )
            nc.vector.tensor_add(cout[:, ct:ct + 1], pc[:], pb4[:, ct:ct + 1])

        # scaled weights (groupnorm 'a' folded in): ws = w * a[channel]
        ws = {}
        for nm in ("q", "k", "v"):
            w = wspool.tile([P, CT * C], dt.bfloat16, tag=f"ws_{nm}", name=f"ws_{nm}")
            for kt in range(CT):
                nc.vector.tensor_scalar_mul(
                    w[:, kt * C:(kt + 1) * C],
                    wsb[nm][:, kt * C:(kt + 1) * C],
                    ab[:, kt:kt + 1],
                )
            ws[nm] = w

        # ---------------- phase B: QKV GEMMs ----------------
        psB = psctx.enter_context(tc.tile_pool(name="psB", bufs=4, space="PSUM"))

        # qT[dtile]: [128, 1024]; copy adds q bias
        qT = []
        for dtile in range(CT):
            q = qpool.tile([P, NQ], dt.bfloat16, name=f"qT{dtile}")
            for nch in range(NQ // 512):
                ps = psB.tile([P, 512], dt.float32, name="psb", tag="psb")
                for kt in range(CT):
                    nc.tensor.matmul(
                        ps[:],
                        lhsT=ws["q"][:, kt * C + dtile * P:kt * C + (dtile + 1) * P],
                        rhs=xt[kt][:, nch * 512:(nch + 1) * 512],
                        start=(kt == 0), stop=(kt == CT - 1),
                    )
                nc.vector.tensor_scalar_add(
                    q[:, nch * 512:(nch + 1) * 512], ps[:], qbias[:, dtile:dtile + 1]
                )
            qT.append(q)

        # kT[dtile]: [128, 4096]
        kT = []
        copy_flip = 0
        for dtile in range(CT):
            k = kpool.tile([P, N], dt.bfloat16, name=f"kT{dtile}")
            for nch in range(N // 512):
                ps = psB.tile([P, 512], dt.float32, name="psb", tag="psb")
                for kt in range(CT):
                    nc.tensor.matmul(
                        ps[:],
                        lhsT=ws["k"][:, kt * C + dtile * P:kt * C + (dtile + 1) * P],
                        rhs=xt[kt][:, nch * 512:(nch + 1) * 512],
                        start=(kt == 0), stop=(kt == CT - 1),
                    )
                dst = k[:, nch * 512:(nch + 1) * 512]
                if copy_flip % 2 == 0:
                    nc.scalar.copy(dst, ps[:])
                else:
                    nc.vector.tensor_copy(dst, ps[:])
                copy_flip += 1
            kT.append(k)

        # v[mt]: [128 (m), 512 (d over all heads)]
        vt = []
        for mt in range(MT):
            v = vpool.tile([P, C], dt.bfloat16, name=f"v{mt}")
            ps = psB.tile([P, 512], dt.float32, name="psb", tag="psb")
            for kt in range(CT):
                nc.tensor.matmul(
                    ps[:],
                    lhsT=xt[kt][:, mt * P:(mt + 1) * P],
                    rhs=ws["v"][:, kt * C:(kt + 1) * C],
                    start=(kt == 0), stop=(kt == CT - 1),
                )
            if copy_flip % 2 == 0:
                nc.scalar.copy(v[:], ps[:])
            else:
                nc.vector.tensor_copy(v[:], ps[:])
            copy_flip += 1
            vt.append(v)

        # ---------------- phase C: attention ----------------
        psctx.close()
        spool = ctx.enter_context(tc.tile_pool(name="spool", bufs=2, space="PSUM"))
        opool = ctx.enter_context(tc.tile_pool(name="opool", bufs=1, space="PSUM"))
        obpool = ctx.enter_context(tc.tile_pool(name="obpool", bufs=1, space="PSUM"))

        # steps: (pair, mt, nch) ; QK emission leads PV by one step for pipelining
        steps = [
            (hp, mt, nch)
            for hp in range(NH // 2)
            for mt in range(MT)
            for nch in range(NQ // 512)
        ]

        oa_tiles = {}
        ob_tiles = {}
        s_tiles = {}

        def emit_qk(idx):
            hp, mt, nch = steps[idx]
            s0 = spool.tile([P, 512], dt.float32, tag="sa", name=f"s{idx}a")
            s1 = spool.tile([P, 512], dt.float32, tag="sb", name=f"s{idx}b")
            kk = kT[hp]
            qq = qT[hp]
            nc.tensor.matmul(
                s0[:],
                lhsT=kk[0:64, mt * P:(mt + 1) * P],
                rhs=qq[0:64, nch * 512:(nch + 1) * 512],
                start=True, stop=True,
            )
            nc.tensor.matmul(
                s1[:],
                lhsT=kk[64:128, mt * P:(mt + 1) * P],
                rhs=qq[64:128, nch * 512:(nch + 1) * 512],
                start=True, stop=True,
            )
            s_tiles[idx] = (s0, s1)

        emit_qk(0)
        onorm = []
        for idx, (hp, mt, nch) in enumerate(steps):
            if mt == 0 and nch == 0:
                oa_tiles[hp] = opool.tile([P, NQ], dt.float32, tag="oa", name=f"oa{hp}")
                ob_tiles[hp] = obpool.tile([P, NQ], dt.float32, tag="ob", name=f"ob{hp}")
            if idx + 1 < len(steps):
                emit_qk(idx + 1)
            s0, s1 = s_tiles.pop(idx)
            # head h0 -> exact exp on ScalarE; head h1 -> int16-Schraudolph on DVE
            e0 = epool.tile([P, 512], dt.bfloat16, name=f"e{idx}a", tag="ea")
            nc.scalar.activation(e0[:], s0[:], ACTF.Exp, scale=0.125)
            e1 = epool.tile([P, 512], dt.int16, name=f"e{idx}b", tag="eb")
            nc.vector.tensor_scalar(
                e1[:], s1[:], SCH_A16 * 0.125, SCH_B16, AOT.mult, AOT.add
            )
            eh0 = e0[:]
            eh1 = e1[:].bitcast(dt.bfloat16)
            oa = oa_tiles[hp]
            ob = ob_tiles[hp]
            h0, h1 = 2 * hp, 2 * hp + 1
            first = mt == 0
            last = mt == MT - 1
            v = vt[mt]
            ncol = slice(nch * 512, (nch + 1) * 512)
            nc.tensor.matmul(
                oa[0:64, ncol], lhsT=v[:, h0 * HS:(h0 + 1) * HS],
                rhs=eh0[:], start=first, stop=last, skip_group_check=True,
            )
            nc.tensor.matmul(
                oa[64:128, ncol], lhsT=v[:, h1 * HS:(h1 + 1) * HS],
                rhs=eh1[:], start=first, stop=last, skip_group_check=True,
            )
            nc.tensor.matmul(
                ob[0:64, ncol], lhsT=ones64[:], rhs=eh0[:],
                start=first, stop=last, skip_group_check=True,
            )
            nc.tensor.matmul(
                ob[64:128, ncol], lhsT=ones64[:], rhs=eh1[:],
                start=first, stop=last, skip_group_check=True,
            )
            if last and nch == NQ // 512 - 1:
                # normalize: o_norm = oA * recip_fast(oB); ~2.5us psum hold
                r = rpool.tile([P, NQ], dt.float32, name=f"r{hp}", tag="r")
                nc.vector.reciprocal_approx_fast(r[:], ob[:])
                on = onpool.tile([P, NQ], dt.bfloat16, name=f"on{hp}")
                nc.vector.tensor_mul(on[:], oa[:], r[:])
                onorm.append(on)
                del oa_tiles[hp], ob_tiles[hp]

        # ---------------- phase D: proj + residual + out ----------------
        for ct in range(CT):
            # reuse the (now-free) oa/ob psum slots, alternating for double-buffer
            ypool = opool if ct % 2 == 0 else obpool
            y = ypool.tile([P, NQ], dt.float32, tag="oa" if ct % 2 == 0 else "ob",
                           name=f"yps{ct}")
            for nch in range(NQ // 512):
                for kt in range(CT):
                    nc.tensor.matmul(
                        y[:, nch * 512:(nch + 1) * 512],
                        lhsT=wsb["p"][:, kt * C + ct * P:kt * C + (ct + 1) * P],
                        rhs=onorm[kt][:, nch * 512:(nch + 1) * 512],
                        start=(kt == 0), stop=(kt == CT - 1),
                    )
            rsd = rsdpool.tile([P, NQ], dt.float32, name=f"rsd{ct}", tag="rsd")
            nc.sync.dma_start(rsd[:], xq_d.ap()[ct * P:(ct + 1) * P, :])
            ot = outpool.tile([P, NQ], dt.float32, name=f"ot{ct}", tag="ot")
            # (y + cout) + resid in one DVE pass
            nc.vector.scalar_tensor_tensor(
                ot[:], y[:], cout[:, ct:ct + 1], rsd[:], AOT.add, AOT.add
            )
            nc.sync.dma_start(y_d.ap()[ct * P:(ct + 1) * P, :], ot[:])

    nc.compile()
    return nc


def make_in_maps(inputs):
    x = np.asarray(inputs["x"], dtype=np.float32).reshape(2, C, N)
    gn_w = np.asarray(inputs["gn_w"], dtype=np.float32)
    gn_b = np.asarray(inputs["gn_b"], dtype=np.float32)

    def t4(v):
        return np.ascontiguousarray(np.asarray(v, np.float32).reshape(CT, P).T)

    wqT = np.ascontiguousarray(np.asarray(inputs["wq_w"], np.float32).T).astype(BF16)
    wkT = np.ascontiguousarray(np.asarray(inputs["wk_w"], np.float32).T).astype(BF16)
    wvT = np.ascontiguousarray(np.asarray(inputs["wv_w"], np.float32).T).astype(BF16)
    wpT = np.ascontiguousarray(np.asarray(inputs["proj_w"], np.float32).T).astype(BF16)
    gnw4 = t4(gn_w)
    gnb4 = t4(gn_b)
    qb4 = t4(inputs["wq_b"])
    wvb4 = t4(inputs["wv_b"])
    pb4 = t4(inputs["proj_b"])
    inda = np.zeros((P, 8), np.float32)
    for p in range(P):
        inda[p, p // 16] = 1.0
    indb = np.ascontiguousarray(inda.T)

    shared = dict(
        wqT=wqT, wkT=wkT, wvT=wvT, wpT=wpT,
        gnw4=gnw4, gnb4=gnb4, qb4=qb4, wvb4=wvb4, pb4=pb4,
        inda=inda, indb=indb,
    )
    in_maps = []
    for core in range(8):
        b, r = core // 4, core % 4
        nq0 = r * NQ
        rolled = np.roll(x[b], -nq0, axis=1)
        m = dict(shared)
        m["xb"] = rolled.astype(BF16)
        m["xq"] = np.ascontiguousarray(x[b][:, nq0:nq0 + NQ])
        in_maps.append(m)
    return in_maps


def assemble(results):
    out = np.empty((2, C, N), np.float32)
    for core in range(8):
        b, r = core // 4, core % 4
        out[b][:, r * NQ:(r + 1) * NQ] = results[core]["y"]
    return out.reshape(2, C, 64, 64)


def get_program():
    if "nc" not in _CACHE:
        _CACHE["nc"] = build_program()
    return _CACHE["nc"]


def kernel(**inputs):
    nc = get_program()
    in_maps = make_in_maps(inputs)
    res = run_bass_kernel_spmd(nc, in_maps, core_ids=list(range(8)))
    return assemble(res.results)
